# revision 66
# baseline (speedup 1.0000x reference)
"""Block-diagonal rotation (COB) kernel for Trainium2, 8 NeuronCores.

Computes out[..., block_i] = x[..., block_i] @ W_i.T for 8 square blocks of
sizes [512, 1024, 256, 768, 384, 640, 128, 384] (features sum to 4096),
x shape (4, 2048, 4096) fp32.

Strategy (bf16 end-to-end, data-parallel over rows, W-stationary):
  - 8192 rows split 8 ways (1024 rows/core); each core holds all weights.
  - The HOST pre-transposes x per core and packs it in PE-consumption
    order; it also unscrambles the outT blocks the device returns.  The
    device never transposes: the PE computes outT[n, m] = sum_d W[n, d]
    * xT[d, m] with 128x128 W chunks stationary and xT streaming 512
    rows per matmul.  328 matmuls/core, all N=512: 167,936 PE cycles =
    70 us at 2.4 GHz -- the bf16 streaming floor for this op (fp8 would
    halve it but its ~2.5e-2 max rel err fails the 2e-2 gate).
  - Loop order is k-OUTER (d-chunk stages) with a block's n-chunk PSUM
    groups accumulating concurrently (j-halved on m0 passes, which need
    no new input, so copies drain while the second half computes), so a
    block's first matmul only needs its first k-tiles -- input demand
    stays smooth, no per-block prefetch cliffs / HAM re-throttles.
  - Per block: m1 row-half pass then m0; block order [6,1,3,5,0,4,7,2]
    starts with tiny b6 (first matmul after ~0.5 MiB of DMA) then big
    blocks first (lowest demand-per-stage while the DMA clock ramps);
    b6's m0 (1 matmul + one 128 KiB store) is held back to run last so
    the kernel drains on the smallest possible tail.
  - 24 warm-up matmuls on a memset scratch tile bridge the DMA-ramp
    window (~7.5-14 us) so the PE HAM clock gate opens before real work
    and never re-throttles; real matmuls then run at the 216 ns/512-col
    issue floor for the whole stream.
  - Input is host-packed into 34 large contiguous DMAs (x: 16 pairs of
    k-tiles [128, 2048] carrying both row-halves; w: 18 per-block
    pair-slices), issued on the sync ring in consumption order.  Each
    dma_start costs ~600 ns of issue time on its engine, so a naive
    107-DMA version was issue-limited to ~210 GB/s input.
  - PSUM results are downcast-copied to bf16 staging (alternating
    ACT/DVE); one output DRAM tensor per (m-half, block) holds the
    staging tile verbatim; the host reassembles.  Bulk stores ride the
    gpsimd SWDGE ring (own flow-control sems, so store completions --
    gated on compute -- never block input issue via the 8 shared HWDGE
    DMA lanes); the last 5 stores use the then-idle scalar ring (SWDGE's
    ~1-2 us/DMA issue was serializing the drain).
  - bf16 end-to-end keeps HBM traffic at 21.1 MiB/core; rel err ~3.9e-3
    vs the 2e-2 gate.  Measured: ~95-97 us HW exec on a healthy device
    (baseline 118-135 us); occasional P0 power-throttled runs (PE at
    2.0 GHz, all matmuls ~280 ns) measure ~100-114 us.
"""

import numpy as np
import ml_dtypes

import concourse.bacc as bacc
import concourse.mybir as mybir
from concourse.tile import TileContext
from concourse.bass_utils import run_bass_kernel_spmd

SIZES = [512, 1024, 256, 768, 384, 640, 128, 384]
OFFS = np.cumsum([0] + SIZES)
N_CORES = 8
ROWS_TOTAL = 4 * 2048
ROWS_PER_CORE = ROWS_TOTAL // N_CORES  # 1024
D = 4096
P = 128
M_SLICE = 512                      # rows per PSUM pass (one fp32 bank)
N_MSL = ROWS_PER_CORE // M_SLICE   # 2
KT = D // P                        # 32 global 128-feature chunks

BF16 = mybir.dt.bfloat16
F32 = mybir.dt.float32

# block processing order: b1 first -- its first k-stage is the
# ramp-gated critical chain, and putting anything ahead of it (even
# b6's 32-KiB w DMA + 0.65-us issue slot) delays it.  Big blocks first
# (lowest input-demand rate early); b6's one-matmul m1 rides near the
# end.  b0 (nk=4) is last: its drain runs j-outer and needs groups long
# enough (~1 us) to cover each previous group's copy + store.
BO = [1, 3, 5, 2, 4, 7, 6, 0]

# k-tile consumption order and pairing for the packed x feed
K_ORDER = []
for _b in BO:
    K_ORDER.extend(range(int(OFFS[_b]) // P, int(OFFS[_b + 1]) // P))
K_POS = {k: i for i, k in enumerate(K_ORDER)}
N_XPAIR = KT // 2  # 16 pair-tiles of 2 k-tiles each

_cache = {}


def build_nc():
    if "nc" in _cache:
        return _cache["nc"]
    nc = bacc.Bacc()
    # x feed: 16 pair-tiles [128, 2048]; pair i = k-tiles K_ORDER[2i..2i+1];
    # within a tile, cols a*1024 + m*512 .. +512 hold k-tile a's m-half rows
    xt_d = nc.declare_dram_parameter("xt", [N_XPAIR * P, 4 * M_SLICE], BF16,
                                     isOutput=False)
    # w feed per block: [128, nk*s]; cols k*s + j*128 .. hold the
    # stationary chunk for (d-chunk k, n-chunk j)
    w_d = [
        nc.declare_dram_parameter(f"w{i}", [P, (s // P) * s], BF16, isOutput=False)
        for i, s in enumerate(SIZES)
    ]
    o_d = {
        (m, b): nc.declare_dram_parameter(
            f"o{m}_{b}", [P, (SIZES[b] // P) * M_SLICE], BF16, isOutput=True
        )
        for m in range(N_MSL)
        for b in range(len(SIZES))
    }

    xt_v = xt_d.rearrange("(i p) c -> i p c", p=P)

    with TileContext(nc) as tc:
        with (
            tc.tile_pool(name="wres", bufs=1) as wres,
            tc.tile_pool(name="xres", bufs=1) as xres,
            tc.tile_pool(name="osb", bufs=1) as osb,
            tc.tile_pool(name="mm", bufs=8, space="PSUM") as mm_p,
        ):
            # --- PE warm-up: dummy matmuls on a memset scratch tile so the
            # HAM clock gate opens during the DMA prologue, before real work
            scr = osb.tile([P, M_SLICE], BF16, tag="warm")
            nc.vector.memset(scr[:], 0)
            wps = mm_p.tile([P, M_SLICE], F32, tag="mm", name="warmps")
            # 10 dummies carry the PE just past the HAM clock-gate flip
            # (~3.4 us of sustained busy from ~7.4): with b1-first, its
            # k0 input lands at ~10 us, so real matmuls take over right
            # as the gate opens -- more dummies were measured to gate the
            # real stream by ~3 us.
            for _ in range(10):
                nc.tensor.matmul(wps[:], scr[:, :P], scr[:], start=True,
                                 stop=True)

            # --- input DMAs: all on the sync ring, w pair-slices
            # interleaved with the x quads per k-stage in consumption
            # order (every matmul needs BOTH the w chunk and the x tile).
            # Stores ride the gpsimd SWDGE ring, which has its own
            # flow-control semaphores, so store completions (gated on
            # compute) never block input issue.
            xtiles = {}
            wtile = {}

            # x pair-units (512 KiB, both m-halves) interleaved with w
            # pair-slices per k-stage in consumption order: a single
            # just-in-time path, with the m0 halves arriving a full pass
            # early
            def emit_xpair(i):
                t = xres.tile([P, 4 * M_SLICE], BF16, tag=f"xp{i}",
                              name="xpt")
                nc.sync.dma_start(out=t[:], in_=xt_v[i])
                xtiles[i] = t

            m1p = {"i": 0}
            for b in BO:
                s = SIZES[b]
                nk = s // P
                g0 = int(OFFS[b]) // P
                wt = wres.tile([P, nk * s], BF16, tag=f"w{b}", name="wtt")
                wtile[b] = wt
                # b1's first two k-segs ship as single 256-KiB DMAs: its
                # first k-stage is the ramp-gated critical path and only
                # needs seg 0
                step0 = 1 if b == 1 else 2
                q = 0
                while q < nk:
                    hi = min(q + (step0 if q < 2 else 2), nk)
                    last_pos = max(K_POS[g0 + k] for k in range(q, hi))
                    while m1p["i"] * 2 <= last_pos:
                        emit_xpair(m1p["i"])
                        m1p["i"] += 1
                    nc.sync.dma_start(out=wt[:, q * s:hi * s],
                                      in_=w_d[b][:, q * s:hi * s])
                    q = hi
            while m1p["i"] < N_XPAIR:
                emit_xpair(m1p["i"])
                m1p["i"] += 1

            def xsl(m, k):
                pos = K_POS[k]
                c0 = (pos % 2) * 2 * M_SLICE + m * M_SLICE
                return xtiles[pos // 2][:, c0:c0 + M_SLICE]

            # --- compute: per block, m1 pass then m0 pass, k-outer ---
            cp = {"i": 0, "s": 0}

            def process(b, m, tail_mode=False, inject=None):
                s = SIZES[b]
                nk = s // P
                g0 = int(OFFS[b]) // P
                if tail_mode:
                    # drain blocks (input long resident): j-OUTER with an
                    # immediate copy + per-chunk store per group, so the
                    # stores pipeline against the remaining matmuls and
                    # almost nothing trails the final matmul
                    for j in range(nk):
                        if inject and j == nk - 1:
                            # b6's m0 (1 matmul + 128 KiB store) slots in
                            # before the final group so its drain chain
                            # runs under the last group's matmuls
                            inject()
                        pj = mm_p.tile([P, M_SLICE], F32, tag="mm",
                                       name="mmps")
                        for k in range(nk):
                            nc.tensor.matmul(
                                pj[:],
                                wtile[b][:, k * s + j * P:k * s + (j + 1) * P],
                                xsl(m, g0 + k),
                                start=(k == 0),
                                stop=(k == nk - 1),
                            )
                        stg = osb.tile([P, M_SLICE], BF16, tag=f"ot{b}_{m}_{j}",
                                       name="ott")
                        # b6's drain copy goes on DVE: the scalar engine is
                        # congested with tail stores right then
                        if b == 6 or cp["i"] % 2 != 0:
                            nc.vector.tensor_copy(stg[:], pj[:])
                        else:
                            nc.scalar.copy(stg[:], pj[:])
                        cp["i"] += 1
                        c0 = j * M_SLICE
                        if m == 0 and j == nk - 1 and b != 6:
                            # very last store: two 64-KiB halves on both
                            # HWDGE rings so the HBM write receipts overlap
                            h = M_SLICE // 2
                            nc.scalar.dma_start(
                                out=o_d[(m, b)][:, c0:c0 + h],
                                in_=stg[:, :h])
                            nc.sync.dma_start(
                                out=o_d[(m, b)][:, c0 + h:c0 + M_SLICE],
                                in_=stg[:, h:])
                        else:
                            eng = nc.sync if (b == 6) else nc.scalar
                            eng.dma_start(
                                out=o_d[(m, b)][:, c0:c0 + M_SLICE],
                                in_=stg[:],
                            )
                    return
                # m1 passes run all n-chunk PSUM groups concurrently so the
                # block's input is consumed one k-stage at a time (smooth
                # DMA demand).  m0 passes consume no new input, so big
                # blocks split their groups into halves of <=4 banks --
                # the first half's copies drain while the second computes,
                # removing the 8-bank rotation cliff at pass boundaries.
                if m == 0 and nk > 4:
                    jgs = [range(0, (nk + 1) // 2), range((nk + 1) // 2, nk)]
                else:
                    jgs = [range(nk)]
                # stage tiles are per (block, m-half): sharing one tile
                # across the two passes makes the m0 copies WAR-wait on the
                # m1 store's completion, which backs up the PSUM rotation
                # and stalls the PE mid-pass when a SWDGE store runs slow
                last = b == 6 and m == 0
                stage = osb.tile([P, nk * M_SLICE], BF16, tag=f"os{b}_{m}")
                for jg in jgs:
                    ps = {}
                    for k in range(nk):
                        for j in jg:
                            if k == 0:
                                ps[j] = mm_p.tile([P, M_SLICE], F32,
                                                  tag="mm", name="mmps")
                            nc.tensor.matmul(
                                ps[j][:],
                                wtile[b][:, k * s + j * P:k * s + (j + 1) * P],
                                xsl(m, g0 + k),
                                start=(k == 0),
                                stop=(k == nk - 1),
                            )
                    for j in jg:
                        dst = stage[:, j * M_SLICE:(j + 1) * M_SLICE]
                        if last or cp["i"] % 2 != 0:
                            # final copy on DVE: the scalar engine is
                            # draining the previous block's stores then
                            nc.vector.tensor_copy(dst, ps[j][:])
                        else:
                            nc.scalar.copy(dst, ps[j][:])
                        cp["i"] += 1
                # stores issued while input is still streaming ride the
                # SWDGE ring (their completions would otherwise freeze
                # input issue via the 8 shared HWDGE DMA lanes); once the
                # input stream is done (~55 us, store #8 onward) they
                # switch to the scalar HWDGE ring, which also pulls the
                # 3-4 us SWDGE teardown DRAIN off the critical path
                if cp["s"] >= 8:
                    nc.scalar.dma_start(out=o_d[(m, b)][:, :], in_=stage[:])
                else:
                    nc.gpsimd.dma_start(out=o_d[(m, b)][:, :], in_=stage[:])
                cp["s"] += 1

            for b in BO:
                tail = b == BO[-1]
                process(b, 1, tail_mode=tail)
                if b != 6:
                    process(
                        b, 0, tail_mode=tail,
                        inject=(lambda: process(6, 0, tail_mode=True))
                        if tail else None,
                    )

    nc.finalize()
    _cache["nc"] = nc
    return nc


def build_in_maps(x, w0, w1, w2, w3, w4, w5, w6, w7):
    x = np.asarray(x, dtype=np.float32).reshape(ROWS_TOTAL, D)
    xb = x.astype(ml_dtypes.bfloat16)
    ws = [w0, w1, w2, w3, w4, w5, w6, w7]
    # w feed: [128, nk*s] with cols k*s.. = W.T rows k*128..(k+1)*128
    wfs = []
    for w in ws:
        s = w.shape[0]
        nk = s // P
        wt = np.ascontiguousarray(np.asarray(w, dtype=np.float32).T).astype(
            ml_dtypes.bfloat16
        )
        wfs.append(
            np.ascontiguousarray(
                wt.reshape(nk, P, s).transpose(1, 0, 2).reshape(P, nk * s)
            )
        )
    korder = np.array(K_ORDER)
    in_maps = []
    for c in range(N_CORES):
        xc = xb[c * ROWS_PER_CORE:(c + 1) * ROWS_PER_CORE]  # [1024, 4096]
        xT = np.ascontiguousarray(xc.T)                      # [4096, 1024]
        tiles = xT.reshape(KT, P, ROWS_PER_CORE)             # [32, 128, 1024]
        # pair i: [2, 128, 1024] -> [128, 2, 1024] -> [128, 2048]
        # pair tile layout [pair, p, slot, m-half, 512]
        xf = (
            tiles[korder]
            .reshape(N_XPAIR, 2, P, 2, M_SLICE)
            .transpose(0, 2, 1, 3, 4)
            .reshape(N_XPAIR * P, 4 * M_SLICE)
        )
        m_ = {"xt": np.ascontiguousarray(xf)}
        for i, wf in enumerate(wfs):
            m_[f"w{i}"] = wf
        in_maps.append(m_)
    return in_maps


def kernel(x, w0, w1, w2, w3, w4, w5, w6, w7):
    nc = build_nc()
    in_maps = build_in_maps(x, w0, w1, w2, w3, w4, w5, w6, w7)
    res = run_bass_kernel_spmd(nc, in_maps, list(range(N_CORES)))
    out = np.empty([ROWS_TOTAL, D], dtype=np.float32)
    for c in range(N_CORES):
        rows = out[c * ROWS_PER_CORE:(c + 1) * ROWS_PER_CORE]
        for m in range(N_MSL):
            for b, s in enumerate(SIZES):
                nk = s // P
                o = res.results[c][f"o{m}_{b}"]  # [128, nk*512]
                # o[p, j*512 + r] = outT[OFFS[b] + j*128 + p, m*512 + r]
                blk = (
                    o.reshape(P, nk, M_SLICE)
                    .transpose(1, 0, 2)
                    .reshape(s, M_SLICE)
                )
                rows[m * M_SLICE:(m + 1) * M_SLICE, OFFS[b]:OFFS[b] + s] = blk.T.astype(
                    np.float32
                )
    return out.reshape(4, 2048, D)
